# revision 30
# baseline (speedup 1.0000x reference)
"""Trainium2 Bass kernel for grouped-query attention with qk-norm.

Problem (hardcoded): x(2,2048,1024) @ Wq(1024,1024) / Wkv(1024,512),
16 query heads, 4 kv heads, head_dim 64, k_scale(16,1,64) applied to the
group-broadcast k. Output (2,2048,1024).

Sharding: 8 cores = batch(2) x kv_heads(4). Each core computes its batch's
4 query heads against its kv head over the full 2048x2048 score matrix.

The kernel is scheduled around the Scalar (ACT) engine: softmax exp over
4 heads x 2048^2 scores is 16.8M activations ~= 143us of ACT busy time,
the per-core critical path. Levers used to stay at/below that floor:
- Scores computed transposed (S^T: keys on partitions) so exp(S^T) feeds
  PV directly as the moving operand; softmax skips max-subtraction
  (inputs bounded) and normalizes after PV via an appended ones-row in V.
- Inputs x/W and the q/k/v activations are fp16 (rel err ~5e-4).
- A subset of exp tiles runs on the Vector engine via a Schraudolph exp2
  bit trick (i16 = round(s*scale*log2e*1024 + 15*1024 - 60), bitcast to
  fp16; rel err ~1.7% on those keys only -> ~6e-3 overall at phi~0.23,
  against a 2e-2 gate), relieving ACT. Extra tiles go to the
  projection-heavy first block where ACT would outpace the PE anyway.
- Host packs x/Wq/Wk/Wv into the exact SBUF layouts so every input DMA
  is contiguous (4KB lines; strided layouts measured ~23GB/s vs ~180+
  contiguous), spread over the three DMA-capable queues JIT.
- Projection chain groups interleave their accumulation matmuls over kt
  so the PE keeps high duty while x streams in (HAM stays warm).
- All transient PSUM (scores, projections, warmups, v-transposes) shares
  one 3-slot ring (6 banks) + 2 banks for the PV accumulators, so a
  Vector-engine exp tile never blocks the Scalar engine's next tile.
- Each 512-query block drains, reciprocals (via a DRAM-bounce respread)
  and writes out inline under the next block's exp; the last block
  normalizes straight from PSUM with a 1-row iterative reciprocal.
"""

import os
from contextlib import ExitStack

import numpy as np

import concourse.bacc as bacc
import concourse.mybir as mybir
import concourse.tile as tile
from concourse.bass_utils import run_bass_kernel_spmd

# Problem constants
B, N, DIM = 2, 2048, 1024
HEADS, KV_HEADS, DH = 16, 4, 64
G = HEADS // KV_HEADS  # query heads per kv head (4)
NCORES = 8
P = 128
KT = DIM // P  # 8 contraction tiles over dim
IC = 512  # query-chunk width
NI = N // IC  # 4
NJ = N // P  # 16 key tiles
NW = NJ // NI  # 4 key tiles per projection wave
SCALE = DH**-0.5

F32 = mybir.dt.float32
F16 = mybir.dt.float16
I16 = mybir.dt.int16

# Schraudolph exp2 offload to the Vector engine. NS0 tiles (of 16) in the
# first block, NS elsewhere.
NSCH0 = int(os.environ.get("KERNEL_NSCH0", "0"))
NSCH = int(os.environ.get("KERNEL_NSCH", "3"))
SCH_C = float(os.environ.get("KERNEL_SCH_C", "60.0"))
SCH_MULT = SCALE * np.log2(np.e) * 1024.0
SCH_ADD = 15.0 * 1024.0 - SCH_C


def emit_kernel(ctx, tc, xp, wq, wk, wv, eye, oT):
    nc = tc.nc
    Exp = mybir.ActivationFunctionType.Exp
    mult = mybir.AluOpType.mult
    add = mybir.AluOpType.add

    wpool = ctx.enter_context(tc.tile_pool(name="w", bufs=1))
    qkpool = ctx.enter_context(tc.tile_pool(name="qk", bufs=1))
    ptpool = ctx.enter_context(tc.tile_pool(name="pt", bufs=6))
    npool = ctx.enter_context(tc.tile_pool(name="norm", bufs=2))
    # PSUM budget (8 banks of 2KB/partition):
    #   shared ring (scores/proj/warmup/vT) 3 x [128,1024] f32 = 6 banks
    #   o_ps (PV accumulators)              2 x [65,512]   f32 = 2 banks
    apsum = ctx.enter_context(tc.tile_pool(name="ap", bufs=3, space="PSUM"))
    opool = ctx.enter_context(tc.tile_pool(name="op", bufs=1, space="PSUM"))

    # --- persistent SBUF tensors ---
    ones_sb = wpool.tile([P, DH], F32, tag="ones")
    eye_sb = wpool.tile([DH, DH], F16, tag="eye")
    qT = [qkpool.tile([P, N], F16, name=f"qT{hp}", tag=f"qT{hp}") for hp in range(2)]
    kkT = [qkpool.tile([P, N], F16, name=f"kkT{hp}", tag=f"kkT{hp}") for hp in range(2)]
    vaug = qkpool.tile([P, NJ * (DH + 1)], F16, tag="vaug")
    wq_sb = qkpool.tile([P, KT * 256], F16, tag="wq")
    wk_sb = qkpool.tile([P, KT * 256], F16, tag="wk")
    wv_sb = qkpool.tile([P, KT * DH], F16, tag="wv")
    # x resident in SBUF, laid out [p, ic, kt, c] so each ic chunk is one
    # contiguous DMA from the host-packed xp.
    xts = qkpool.tile([P, KT * N], F16, tag="xt")  # 4MB
    vT_sb = qkpool.tile([DH, N], F16, tag="vT")

    def xsl(ic, kt):
        base = ic * KT * IC + kt * IC
        return xts[:, base : base + IC]

    nc.any.memset(vaug[:], 1.0)
    nc.any.memset(ones_sb[:], 1.0)
    warm = qkpool.tile([1, 1], F32, tag="warm")
    nc.scalar.activation(warm[:], ones_sb[0:1, 0:1], Exp)

    sums_d = nc.dram_tensor("sums_d", (G, N), F32, kind="ExternalOutput").ap()
    rec_d = nc.dram_tensor("rec_d", (G, N), F32, kind="ExternalOutput").ap()
    o_acc = [
        npool.tile([DH + 1, N], F32, name=f"oacc{h}", tag=f"oacc{h}", bufs=1)
        for h in range(G)
    ]
    rec_row = [
        npool.tile([1, N], F32, name=f"recrow{h}", tag=f"recrow{h}", bufs=1)
        for h in range(G)
    ]

    # --- input DMAs: all fully contiguous (host-packed), spread JIT ---
    #   gpsimd: x-ic0 (two halves, gates first kk/q chains), x-ic2
    #   sync:   wk, x-ic1, x-ic3
    #   scalar: wq, wv  (scalar queue frees well before the first exp)
    XC = KT * IC  # 4096 columns per ic chunk
    nc.sync.dma_start(eye_sb[:], eye[:, :])
    # Queues sustain only ~55GB/s each; chunk the first block's inputs so
    # the kk/q chains pipeline with arrival instead of waiting for a
    # single completion semaphore.
    HW = KT * 128  # half of a packed weight
    nc.sync.dma_start(wk_sb[:, 0:HW], wk[:, 0:HW])
    nc.scalar.dma_start(wq_sb[:, 0:HW], wq[:, 0:HW])
    nc.gpsimd.dma_start(xts[:, 0 : XC // 4], xp[:, 0 : XC // 4])
    nc.sync.dma_start(wk_sb[:, HW:], wk[:, HW:])
    nc.scalar.dma_start(wq_sb[:, HW:], wq[:, HW:])
    nc.gpsimd.dma_start(xts[:, XC // 4 : XC // 2], xp[:, XC // 4 : XC // 2])
    nc.scalar.dma_start(xts[:, XC // 2 : 3 * XC // 4], xp[:, XC // 2 : 3 * XC // 4])
    nc.sync.dma_start(xts[:, 3 * XC // 4 : XC], xp[:, 3 * XC // 4 : XC])
    nc.scalar.dma_start(wv_sb[:], wv[:, :])
    nc.gpsimd.dma_start(xts[:, XC : 2 * XC], xp[:, XC : 2 * XC])
    nc.sync.dma_start(xts[:, 2 * XC : 3 * XC], xp[:, 2 * XC : 3 * XC])
    nc.gpsimd.dma_start(xts[:, 3 * XC : 4 * XC], xp[:, 3 * XC : 4 * XC])

    # Dummy matmuls during the initial DMA wait keep the PE HAM activity
    # monitor busy so real projections start at 2.4GHz instead of 1.2.
    for _ in range(32):
        wt = apsum.tile([DH, IC], F32, tag="s", name="wt")
        nc.tensor.matmul(
            wt[:, 0:DH], ones_sb[:, 0:DH], ones_sb[:, 0:DH], start=True, stop=True
        )

    # --- projection waves (emitted JIT inside the attention loop) ---
    # A group of chains interleaves its matmuls over kt so the PE keeps
    # high duty while x chunks stream in.
    def proj_group(chains, ic, pool=None, tag="s"):
        csl = slice(ic * IC, (ic + 1) * IC)
        pool = pool or apsum
        pss = [
            pool.tile([rows, IC], F32, tag=tag, name="pjt")
            for (dst, w_sb, c0, rows) in chains
        ]
        for kt in range(KT):
            for (dst, w_sb, c0, rows), ps in zip(chains, pss):
                nc.tensor.matmul(
                    ps[:],
                    w_sb[:, kt * 256 + c0 : kt * 256 + c0 + rows]
                    if rows == P
                    else w_sb[:, kt * DH : (kt + 1) * DH],
                    xsl(ic, kt),
                    start=(kt == 0),
                    stop=(kt == KT - 1),
                )
        for (dst, w_sb, c0, rows), ps in zip(chains, pss):
            nc.vector.tensor_copy(dst[:, csl], ps[:])

    def kk_wave(ic, hps):
        proj_group([(kkT[hp], wk_sb, hp * 128, P) for hp in hps], ic)

    def q_wave(ic, hps):
        proj_group([(qT[hp], wq_sb, hp * 128, P) for hp in hps], ic)

    def v_wave(ic):
        proj_group([(vT_sb, wv_sb, 0, DH)], ic)
        # all 4 key-tile transposes into one psum tile, one strided copy out
        pv = apsum.tile([P, NW, DH], F16, tag="s", name="pvt")
        for w in range(NW):
            jt = NW * ic + w
            nc.tensor.transpose(
                pv[:, w, :], vT_sb[:, jt * P : (jt + 1) * P], eye_sb[:]
            )
        dst = vaug[:, NW * ic * (DH + 1) : (NW * ic + NW) * (DH + 1)]
        nc.vector.tensor_copy(
            dst.rearrange("p (w c) -> p w c", c=DH + 1)[:, :, 0:DH], pv[:]
        )

    # --- attention primitives ---
    def qk_mm(hp, ic, jt):
        csl = slice(ic * IC, (ic + 1) * IC)
        st = apsum.tile([P, 2 * IC], F32, tag="s", name="st")
        for half in range(2):
            rsl = slice(half * 64, half * 64 + 64)
            nc.tensor.matmul(
                st[:, half * IC : (half + 1) * IC],
                kkT[hp][rsl, jt * P : (jt + 1) * P],
                qT[hp][rsl, csl],
                start=True,
                stop=True,
                tile_position=(half * 64, 0),
            )
        return st

    def exp_tile(st, use_dve):
        pt = ptpool.tile([P, 2 * IC], F16, tag="pt")
        if use_dve:
            nc.vector.tensor_scalar(
                pt[:].bitcast(I16), st[:], SCH_MULT, SCH_ADD, mult, add
            )
        else:
            nc.scalar.activation(pt[:], st[:], Exp, scale=SCALE)
        return pt

    def pv_mm(o_ps, jt, pt):
        for half in range(2):
            nc.tensor.matmul(
                o_ps[:, half * IC : (half + 1) * IC],
                vaug[:, jt * (DH + 1) : (jt + 1) * (DH + 1)],
                pt[:, half * IC : (half + 1) * IC],
                start=(jt == 0),
                stop=(jt == NJ - 1),
            )

    def recip_chunk(h, ic):
        # DVE reciprocal on a 1-row AP is iterative (~8cyc/elem on one
        # lane); on (128,4) it is ~100x cheaper. The sums row is respread
        # across partitions via a DRAM bounce (DMA cannot repartition
        # within SBUF). Latency hides under the next block.
        csl = slice(ic * IC, (ic + 1) * IC)
        sums_t = npool.tile([P, 4], F32, tag="sums_t", bufs=2)
        rec_t = npool.tile([P, 4], F32, tag="rec_t", bufs=2)
        nc.sync.dma_start(sums_d[h : h + 1, csl], o_acc[h][DH : DH + 1, csl])
        nc.sync.dma_start(
            sums_t[:], sums_d[h : h + 1, csl].rearrange("o (p f) -> (o p) f", p=P)
        )
        nc.vector.reciprocal(rec_t[:], sums_t[:])
        nc.sync.dma_start(
            rec_d[h : h + 1, csl].rearrange("o (p f) -> (o p) f", p=P), rec_t[:]
        )
        nc.sync.dma_start(rec_row[h][0:1, csl], rec_d[h : h + 1, csl])

    def normalize_chunk(h, ic, src):
        # GpSimd broadcasts the reciprocal row across partitions (PE-free).
        csl = slice(ic * IC, (ic + 1) * IC)
        bc = npool.tile([DH, IC], F32, name="bcg", tag="bcg", bufs=4)
        nc.gpsimd.partition_broadcast(bc[:], rec_row[h][0:1, csl])
        fin = npool.tile([DH, IC], F32, tag="fin", bufs=4)
        nc.vector.tensor_tensor(fin[:], src, bc[:], mult)
        nc.sync.dma_start(oT[h * DH : (h + 1) * DH, csl], fin[:])

    def drain_block(hp, ic, o_ps, last):
        csl = slice(ic * IC, (ic + 1) * IC)
        if last:
            # Shortest exposed tail: iterative reciprocal directly on the
            # 1-row psum sums (~3.3us each on one lane, but no 4-hop DRAM
            # bounce latency), normalize straight from PSUM.
            for half in range(2):
                h = 2 * hp + half
                nc.vector.reciprocal(
                    rec_row[h][0:1, csl],
                    o_ps[DH : DH + 1, half * IC : (half + 1) * IC],
                )
                normalize_chunk(h, ic, o_ps[0:DH, half * IC : (half + 1) * IC])
        else:
            for half in range(2):
                h = 2 * hp + half
                nc.vector.tensor_copy(
                    o_acc[h][:, csl], o_ps[:, half * IC : (half + 1) * IC]
                )
                recip_chunk(h, ic)
                normalize_chunk(h, ic, o_acc[h][0:DH, csl])

    def sch_set(n):
        if n <= 0:
            return set()
        step = NJ // n
        return {NJ - 1 - i * step for i in range(n)}

    # --- main loop: ACT-centric pipeline ---
    # Block (0,0) carries the projection waves JIT: kk chunk w must precede
    # QK of j-tiles 4w.., v chunk w must precede PV of j-tile 4w. hp=1
    # projections are deferred into later blocks' PE slack.
    proj_group([(kkT[0], wk_sb, 0, P), (qT[0], wq_sb, 0, P)], 0)
    for hp in range(2):
        for ic in range(NI):
            first_block = hp == 0 and ic == 0
            last_block = hp == 1 and ic == NI - 1
            o_ps = opool.tile([DH + 1, 2 * IC], F32, name="ops", tag="ops", bufs=1)
            sch = sch_set(NSCH0 if first_block else NSCH)
            if last_block and sch:
                # keep the final exps on ACT so the tail isn't gated on
                # the slower Vector-engine exp
                sch = {jt - 3 for jt in sch}
            pend = []  # (jt, pt) awaiting PV
            for jt in range(NJ):
                if first_block:
                    if jt % NW == 0 and jt > 0:
                        kk_wave(jt // NW, [0])
                    if jt % NW == 2:
                        kk_wave(jt // NW, [1])
                st = qk_mm(hp, ic, jt)
                if first_block and jt % NW == 0:
                    v_wave(jt // NW)
                if jt == 8 and (hp, ic) != (1, NI - 1):
                    # next block's q projection, mid-block so the score
                    # pipeline absorbs it instead of stalling at the
                    # block boundary
                    nhp, nic = (hp, ic + 1) if ic < NI - 1 else (1, 0)
                    q_wave(nic, [nhp])
                pend.append((jt, exp_tile(st, jt in sch)))
                if len(pend) > 1:
                    j0, pt0 = pend.pop(0)
                    pv_mm(o_ps, j0, pt0)
            for j0, pt0 in pend:
                pv_mm(o_ps, j0, pt0)
            drain_block(
                hp,
                ic,
                o_ps,
                last=(hp == 1 and ic == NI - 1)
                and os.environ.get("KERNEL_FASTTAIL", "1") == "1",
            )


_CACHE = {}


def _enable_ldw_opt():
    # The concourse walrus wrapper hardcodes --enable-ldw-opt=false; with it
    # off every matmul pays an exposed LDWEIGHTS (~120ns x ~560 matmuls).
    # Opt in to the optimized weight-load path for this kernel; correctness
    # is checked end-to-end against the reference output.
    # Tried: breaks walrus codegen (visitInstLdweights assertion) in this
    # compiler build — that is why concourse pins it false. Off by default.
    if os.environ.get("KERNEL_LDW_OPT", "0") != "1":
        return
    import concourse.bass_utils as bu

    orig = bu.run_command
    if getattr(orig, "_ldw_patched", False):
        return

    def run_command_ldw(cmd, *a, **kw):
        if isinstance(cmd, list):
            cmd = [
                "--enable-ldw-opt=true" if c == "--enable-ldw-opt=false" else c
                for c in cmd
            ]
        return orig(cmd, *a, **kw)

    run_command_ldw._ldw_patched = True
    bu.run_command = run_command_ldw


def build():
    if "nc" in _CACHE:
        return _CACHE["nc"]
    _enable_ldw_opt()
    nc = bacc.Bacc(
        "TRN2", target_bir_lowering=False, debug=False, num_devices=NCORES
    )
    xp = nc.dram_tensor("xp", (P, KT * N), F16, kind="ExternalInput").ap()
    wq = nc.dram_tensor("wq", (P, KT * 256), F16, kind="ExternalInput").ap()
    wk = nc.dram_tensor("wk", (P, KT * 256), F16, kind="ExternalInput").ap()
    wv = nc.dram_tensor("wv", (P, KT * DH), F16, kind="ExternalInput").ap()
    eye = nc.dram_tensor("eye", (DH, DH), F16, kind="ExternalInput").ap()
    oT = nc.dram_tensor("oT", (G * DH, N), F32, kind="ExternalOutput").ap()
    with tile.TileContext(nc) as tc:
        with ExitStack() as ctx:
            emit_kernel(ctx, tc, xp, wq, wk, wv, eye, oT)
    nc.compile()
    _CACHE["nc"] = nc
    return nc


def _pack_w(w):
    # (KT*128, width) -> sbuf layout [p, kt*width]
    kt, width = w.shape[0] // P, w.shape[1]
    return np.ascontiguousarray(
        w.reshape(kt, P, width).transpose(1, 0, 2).reshape(P, kt * width)
    )


def make_in_maps(x, Wq, Wkv, k_scale):
    x = np.asarray(x, dtype=np.float32)
    Wq = np.asarray(Wq, dtype=np.float32)
    Wkv = np.asarray(Wkv, dtype=np.float32)
    k_scale = np.asarray(k_scale, dtype=np.float32)
    # x packed to [p, ic, kt, c]: contiguous per-ic-chunk DMAs.
    xps = []
    for b in range(B):
        xT = x[b].T.astype(np.float16)  # (1024, 2048)
        xp = (
            xT.reshape(KT, P, NI, IC)
            .transpose(1, 2, 0, 3)
            .reshape(P, KT * N)
        )
        xps.append(np.ascontiguousarray(xp))
    in_maps = []
    for c in range(NCORES):
        b, kv = divmod(c, KV_HEADS)
        wk_base = Wkv[:, kv * DH : (kv + 1) * DH]
        wk_c = np.concatenate(
            [wk_base * k_scale[kv * G + j, 0][None, :] for j in range(G)], axis=1
        )
        in_maps.append(
            {
                "xp": xps[b],
                "wq": _pack_w(
                    Wq[:, kv * G * DH : (kv + 1) * G * DH].astype(np.float16)
                ),
                "wk": _pack_w(wk_c.astype(np.float16)),
                "wv": _pack_w(
                    Wkv[
                        :, KV_HEADS * DH + kv * DH : KV_HEADS * DH + (kv + 1) * DH
                    ].astype(np.float16)
                ),
                "eye": np.eye(DH, dtype=np.float16),
            }
        )
    return in_maps


def gather(results):
    out = np.empty((B, N, HEADS * DH), dtype=np.float32)
    for c in range(NCORES):
        b, kv = divmod(c, KV_HEADS)
        out[b, :, kv * G * DH : (kv + 1) * G * DH] = results[c]["oT"].T
    return out


def kernel(x, Wq, Wkv, k_scale, _trace=False):
    nc = build()
    in_maps = make_in_maps(x, Wq, Wkv, k_scale)
    res = run_bass_kernel_spmd(
        nc, in_maps, core_ids=list(range(NCORES)), trace=_trace
    )
    out = gather(res.results)
    if _trace:
        kernel.last_result = res
    return out


# revision 31
# speedup vs baseline: 1.2068x; 1.2068x over previous
"""Trainium2 Bass kernel for grouped-query attention with qk-norm.

Problem (hardcoded): x(2,2048,1024) @ Wq(1024,1024) / Wkv(1024,512),
16 query heads, 4 kv heads, head_dim 64, k_scale(16,1,64) applied to the
group-broadcast k. Output (2,2048,1024).

Sharding: 8 cores = batch(2) x kv_heads(4). Each core computes its batch's
4 query heads against its kv head over the full 2048x2048 score matrix.

The kernel is scheduled around the Scalar (ACT) engine: softmax exp over
4 heads x 2048^2 scores is 16.8M activations ~= 143us of ACT busy time,
the per-core critical path. Levers used to stay at/below that floor:
- Scores computed transposed (S^T: keys on partitions) so exp(S^T) feeds
  PV directly as the moving operand; softmax skips max-subtraction
  (inputs bounded) and normalizes after PV via an appended ones-row in V.
- Inputs x/W and the q/k/v activations are fp16 (rel err ~5e-4).
- A subset of exp tiles runs on the Vector engine via a Schraudolph exp2
  bit trick (i16 = round(s*scale*log2e*1024 + 15*1024 - 60), bitcast to
  fp16; rel err ~1.7% on those keys only -> ~6e-3 overall at phi~0.23,
  against a 2e-2 gate), relieving ACT. Extra tiles go to the
  projection-heavy first block where ACT would outpace the PE anyway.
- Host packs x/Wq/Wk/Wv into the exact SBUF layouts so every input DMA
  is contiguous (4KB lines; strided layouts measured ~23GB/s vs ~180+
  contiguous), spread over the three DMA-capable queues JIT.
- Projection chain groups interleave their accumulation matmuls over kt
  so the PE keeps high duty while x streams in (HAM stays warm).
- All transient PSUM (scores, projections, warmups, v-transposes) shares
  one 3-slot ring (6 banks) + 2 banks for the PV accumulators, so a
  Vector-engine exp tile never blocks the Scalar engine's next tile.
- Each 512-query block drains, reciprocals (via a DRAM-bounce respread)
  and writes out inline under the next block's exp; the last block
  normalizes straight from PSUM with a 1-row iterative reciprocal.
"""

import os
from contextlib import ExitStack

import numpy as np

import concourse.bacc as bacc
import concourse.mybir as mybir
import concourse.tile as tile
from concourse.bass_utils import run_bass_kernel_spmd

# Problem constants
B, N, DIM = 2, 2048, 1024
HEADS, KV_HEADS, DH = 16, 4, 64
G = HEADS // KV_HEADS  # query heads per kv head (4)
NCORES = 8
P = 128
KT = DIM // P  # 8 contraction tiles over dim
IC = 512  # query-chunk width
NI = N // IC  # 4
NJ = N // P  # 16 key tiles
NW = NJ // NI  # 4 key tiles per projection wave
SCALE = DH**-0.5

F32 = mybir.dt.float32
F16 = mybir.dt.float16
I16 = mybir.dt.int16

# Schraudolph exp2 offload to the Vector engine. NS0 tiles (of 16) in the
# first block, NS elsewhere.
NSCH0 = int(os.environ.get("KERNEL_NSCH0", "0"))
NSCH = int(os.environ.get("KERNEL_NSCH", "3"))
SCH_C = float(os.environ.get("KERNEL_SCH_C", "60.0"))
SCH_MULT = SCALE * np.log2(np.e) * 1024.0
SCH_ADD = 15.0 * 1024.0 - SCH_C


def emit_kernel(ctx, tc, xp, wq, wk, wv, eye, oT):
    nc = tc.nc
    Exp = mybir.ActivationFunctionType.Exp
    mult = mybir.AluOpType.mult
    add = mybir.AluOpType.add

    wpool = ctx.enter_context(tc.tile_pool(name="w", bufs=1))
    qkpool = ctx.enter_context(tc.tile_pool(name="qk", bufs=1))
    ptpool = ctx.enter_context(tc.tile_pool(name="pt", bufs=6))
    npool = ctx.enter_context(tc.tile_pool(name="norm", bufs=2))
    # PSUM budget (8 banks of 2KB/partition):
    #   shared ring (scores/proj/warmup/vT) 3 x [128,1024] f32 = 6 banks
    #   o_ps (PV accumulators)              2 x [65,512]   f32 = 2 banks
    apsum = ctx.enter_context(tc.tile_pool(name="ap", bufs=3, space="PSUM"))
    opool = ctx.enter_context(tc.tile_pool(name="op", bufs=1, space="PSUM"))

    # --- persistent SBUF tensors ---
    ones_sb = wpool.tile([P, DH], F32, tag="ones")
    eye_sb = wpool.tile([DH, DH], F16, tag="eye")
    qT = [qkpool.tile([P, N], F16, name=f"qT{hp}", tag=f"qT{hp}") for hp in range(2)]
    kkT = [qkpool.tile([P, N], F16, name=f"kkT{hp}", tag=f"kkT{hp}") for hp in range(2)]
    vaug = qkpool.tile([P, NJ * (DH + 1)], F16, tag="vaug")
    wq_sb = qkpool.tile([P, KT * 256], F16, tag="wq")
    wk_sb = qkpool.tile([P, KT * 256], F16, tag="wk")
    wv_sb = qkpool.tile([P, KT * DH], F16, tag="wv")
    # x resident in SBUF, laid out [p, ic, kt, c] so each ic chunk is one
    # contiguous DMA from the host-packed xp.
    xts = qkpool.tile([P, KT * N], F16, tag="xt")  # 4MB
    vT_sb = qkpool.tile([DH, N], F16, tag="vT")

    def xsl(ic, kt):
        base = ic * KT * IC + kt * IC
        return xts[:, base : base + IC]

    nc.any.memset(vaug[:], 1.0)
    nc.any.memset(ones_sb[:], 1.0)
    warm = qkpool.tile([1, 1], F32, tag="warm")
    nc.scalar.activation(warm[:], ones_sb[0:1, 0:1], Exp)

    sums_d = nc.dram_tensor("sums_d", (G, N), F32, kind="ExternalOutput").ap()
    rec_d = nc.dram_tensor("rec_d", (G, N), F32, kind="ExternalOutput").ap()
    o_acc = [
        npool.tile([DH + 1, N], F32, name=f"oacc{h}", tag=f"oacc{h}", bufs=1)
        for h in range(G)
    ]
    rec_row = [
        npool.tile([1, N], F32, name=f"recrow{h}", tag=f"recrow{h}", bufs=1)
        for h in range(G)
    ]

    # --- input DMAs: all fully contiguous (host-packed), spread JIT ---
    #   gpsimd: x-ic0 (two halves, gates first kk/q chains), x-ic2
    #   sync:   wk, x-ic1, x-ic3
    #   scalar: wq, wv  (scalar queue frees well before the first exp)
    XC = KT * IC  # 4096 columns per ic chunk
    nc.sync.dma_start(eye_sb[:], eye[:, :])
    # Queues sustain only ~55GB/s each; chunk the first block's inputs so
    # the kk/q chains pipeline with arrival instead of waiting for a
    # single completion semaphore.
    HW = KT * 128  # half of a packed weight
    nc.sync.dma_start(wk_sb[:, 0:HW], wk[:, 0:HW])
    nc.scalar.dma_start(wq_sb[:, 0:HW], wq[:, 0:HW])
    nc.gpsimd.dma_start(xts[:, 0 : XC // 4], xp[:, 0 : XC // 4])
    nc.sync.dma_start(wk_sb[:, HW:], wk[:, HW:])
    nc.scalar.dma_start(wq_sb[:, HW:], wq[:, HW:])
    nc.gpsimd.dma_start(xts[:, XC // 4 : XC // 2], xp[:, XC // 4 : XC // 2])
    nc.scalar.dma_start(xts[:, XC // 2 : 3 * XC // 4], xp[:, XC // 2 : 3 * XC // 4])
    nc.sync.dma_start(xts[:, 3 * XC // 4 : XC], xp[:, 3 * XC // 4 : XC])
    nc.scalar.dma_start(wv_sb[:], wv[:, :])
    nc.gpsimd.dma_start(xts[:, XC : 2 * XC], xp[:, XC : 2 * XC])
    nc.sync.dma_start(xts[:, 2 * XC : 3 * XC], xp[:, 2 * XC : 3 * XC])
    nc.gpsimd.dma_start(xts[:, 3 * XC : 4 * XC], xp[:, 3 * XC : 4 * XC])

    # Dummy matmuls during the initial DMA wait keep the PE HAM activity
    # monitor busy so real projections start at 2.4GHz instead of 1.2.
    for _ in range(32):
        wt = apsum.tile([DH, IC], F32, tag="s", name="wt")
        nc.tensor.matmul(
            wt[:, 0:DH], ones_sb[:, 0:DH], ones_sb[:, 0:DH], start=True, stop=True
        )

    # --- projection waves (emitted JIT inside the attention loop) ---
    # A group of chains interleaves its matmuls over kt so the PE keeps
    # high duty while x chunks stream in.
    def proj_group(chains, ic, pool=None, tag="s"):
        csl = slice(ic * IC, (ic + 1) * IC)
        pool = pool or apsum
        pss = [
            pool.tile([rows, IC], F32, tag=tag, name="pjt")
            for (dst, w_sb, c0, rows) in chains
        ]
        for kt in range(KT):
            for (dst, w_sb, c0, rows), ps in zip(chains, pss):
                nc.tensor.matmul(
                    ps[:],
                    w_sb[:, kt * 256 + c0 : kt * 256 + c0 + rows]
                    if rows == P
                    else w_sb[:, kt * DH : (kt + 1) * DH],
                    xsl(ic, kt),
                    start=(kt == 0),
                    stop=(kt == KT - 1),
                )
        for (dst, w_sb, c0, rows), ps in zip(chains, pss):
            nc.vector.tensor_copy(dst[:, csl], ps[:])

    def kk_wave(ic, hps):
        proj_group([(kkT[hp], wk_sb, hp * 128, P) for hp in hps], ic)

    def q_wave(ic, hps):
        proj_group([(qT[hp], wq_sb, hp * 128, P) for hp in hps], ic)

    def v_wave(ic):
        proj_group([(vT_sb, wv_sb, 0, DH)], ic)
        # all 4 key-tile transposes into one psum tile, one strided copy out
        pv = apsum.tile([P, NW, DH], F16, tag="s", name="pvt")
        for w in range(NW):
            jt = NW * ic + w
            nc.tensor.transpose(
                pv[:, w, :], vT_sb[:, jt * P : (jt + 1) * P], eye_sb[:]
            )
        dst = vaug[:, NW * ic * (DH + 1) : (NW * ic + NW) * (DH + 1)]
        nc.vector.tensor_copy(
            dst.rearrange("p (w c) -> p w c", c=DH + 1)[:, :, 0:DH], pv[:]
        )

    # --- attention primitives ---
    def qk_mm(hp, ic, jt):
        csl = slice(ic * IC, (ic + 1) * IC)
        st = apsum.tile([P, 2 * IC], F32, tag="s", name="st")
        for half in range(2):
            rsl = slice(half * 64, half * 64 + 64)
            nc.tensor.matmul(
                st[:, half * IC : (half + 1) * IC],
                kkT[hp][rsl, jt * P : (jt + 1) * P],
                qT[hp][rsl, csl],
                start=True,
                stop=True,
                tile_position=(half * 64, 0),
            )
        return st

    def exp_tile(st, use_dve):
        pt = ptpool.tile([P, 2 * IC], F16, tag="pt")
        if use_dve:
            nc.vector.tensor_scalar(
                pt[:].bitcast(I16), st[:], SCH_MULT, SCH_ADD, mult, add
            )
        else:
            nc.scalar.activation(pt[:], st[:], Exp, scale=SCALE)
        return pt

    def pv_mm(o_ps, jt, pt):
        for half in range(2):
            nc.tensor.matmul(
                o_ps[:, half * IC : (half + 1) * IC],
                vaug[:, jt * (DH + 1) : (jt + 1) * (DH + 1)],
                pt[:, half * IC : (half + 1) * IC],
                start=(jt == 0),
                stop=(jt == NJ - 1),
            )

    def recip_chunk(h, ic):
        # DVE reciprocal on a 1-row AP is iterative (~8cyc/elem on one
        # lane); on (128,4) it is ~100x cheaper. The sums row is respread
        # across partitions via a DRAM bounce (DMA cannot repartition
        # within SBUF). Latency hides under the next block.
        csl = slice(ic * IC, (ic + 1) * IC)
        sums_t = npool.tile([P, 4], F32, tag="sums_t", bufs=2)
        rec_t = npool.tile([P, 4], F32, tag="rec_t", bufs=2)
        nc.sync.dma_start(sums_d[h : h + 1, csl], o_acc[h][DH : DH + 1, csl])
        nc.sync.dma_start(
            sums_t[:], sums_d[h : h + 1, csl].rearrange("o (p f) -> (o p) f", p=P)
        )
        nc.vector.reciprocal(rec_t[:], sums_t[:])
        nc.sync.dma_start(
            rec_d[h : h + 1, csl].rearrange("o (p f) -> (o p) f", p=P), rec_t[:]
        )
        nc.sync.dma_start(rec_row[h][0:1, csl], rec_d[h : h + 1, csl])

    def normalize_chunk(h, ic, src):
        # GpSimd broadcasts the reciprocal row across partitions (PE-free).
        csl = slice(ic * IC, (ic + 1) * IC)
        bc = npool.tile([DH, IC], F32, name="bcg", tag="bcg", bufs=4)
        nc.gpsimd.partition_broadcast(bc[:], rec_row[h][0:1, csl])
        fin = npool.tile([DH, IC], F32, tag="fin", bufs=4)
        nc.vector.tensor_tensor(fin[:], src, bc[:], mult)
        nc.sync.dma_start(oT[h * DH : (h + 1) * DH, csl], fin[:])

    def drain_block(hp, ic, o_ps, last):
        csl = slice(ic * IC, (ic + 1) * IC)
        if last:
            # Shortest exposed tail: iterative reciprocal directly on the
            # 1-row psum sums (~3.3us each on one lane, but no 4-hop DRAM
            # bounce latency), normalize straight from PSUM.
            for half in range(2):
                h = 2 * hp + half
                nc.vector.reciprocal(
                    rec_row[h][0:1, csl],
                    o_ps[DH : DH + 1, half * IC : (half + 1) * IC],
                )
                normalize_chunk(h, ic, o_ps[0:DH, half * IC : (half + 1) * IC])
        else:
            for half in range(2):
                h = 2 * hp + half
                nc.vector.tensor_copy(
                    o_acc[h][:, csl], o_ps[:, half * IC : (half + 1) * IC]
                )
                recip_chunk(h, ic)
                normalize_chunk(h, ic, o_acc[h][0:DH, csl])

    def sch_set(n):
        if n <= 0:
            return set()
        step = NJ // n
        return {NJ - 1 - i * step for i in range(n)}

    # --- main loop: ACT-centric pipeline ---
    # Block (0,0) carries the projection waves JIT: kk chunk w must precede
    # QK of j-tiles 4w.., v chunk w must precede PV of j-tile 4w. hp=1
    # projections are deferred into later blocks' PE slack.
    proj_group([(kkT[0], wk_sb, 0, P), (qT[0], wq_sb, 0, P)], 0)
    for hp in range(2):
        for ic in range(NI):
            first_block = hp == 0 and ic == 0
            last_block = hp == 1 and ic == NI - 1
            o_ps = opool.tile([DH + 1, 2 * IC], F32, name="ops", tag="ops", bufs=1)
            sch = sch_set(NSCH0 if first_block else NSCH)
            if last_block and sch:
                # keep the final exps on ACT so the tail isn't gated on
                # the slower Vector-engine exp
                sch = {jt - 3 for jt in sch}
            pend = []  # (jt, pt) awaiting PV
            for jt in range(NJ):
                if first_block:
                    if jt % NW == 0 and jt > 0:
                        kk_wave(jt // NW, [0])
                    if jt == 2:
                        kk_wave(0, [1])
                st = qk_mm(hp, ic, jt)
                if first_block and jt % NW == 0:
                    v_wave(jt // NW)
                if jt == 8 and (hp, ic) != (1, NI - 1):
                    # next block's q projection, mid-block so the score
                    # pipeline absorbs it instead of stalling at the
                    # block boundary
                    nhp, nic = (hp, ic + 1) if ic < NI - 1 else (1, 0)
                    q_wave(nic, [nhp])
                if jt == 12 and hp == 0 and 0 < ic:
                    # spread the deferred hp=1 kk projections over blocks
                    # (0,1)-(0,3) instead of overloading the first block
                    kk_wave(ic, [1])
                pend.append((jt, exp_tile(st, jt in sch)))
                if len(pend) > 1:
                    j0, pt0 = pend.pop(0)
                    pv_mm(o_ps, j0, pt0)
            for j0, pt0 in pend:
                pv_mm(o_ps, j0, pt0)
            drain_block(
                hp,
                ic,
                o_ps,
                last=(hp == 1 and ic == NI - 1)
                and os.environ.get("KERNEL_FASTTAIL", "1") == "1",
            )


_CACHE = {}


def _enable_ldw_opt():
    # The concourse walrus wrapper hardcodes --enable-ldw-opt=false; with it
    # off every matmul pays an exposed LDWEIGHTS (~120ns x ~560 matmuls).
    # Opt in to the optimized weight-load path for this kernel; correctness
    # is checked end-to-end against the reference output.
    # Tried: breaks walrus codegen (visitInstLdweights assertion) in this
    # compiler build — that is why concourse pins it false. Off by default.
    if os.environ.get("KERNEL_LDW_OPT", "0") != "1":
        return
    import concourse.bass_utils as bu

    orig = bu.run_command
    if getattr(orig, "_ldw_patched", False):
        return

    def run_command_ldw(cmd, *a, **kw):
        if isinstance(cmd, list):
            cmd = [
                "--enable-ldw-opt=true" if c == "--enable-ldw-opt=false" else c
                for c in cmd
            ]
        return orig(cmd, *a, **kw)

    run_command_ldw._ldw_patched = True
    bu.run_command = run_command_ldw


def build():
    if "nc" in _CACHE:
        return _CACHE["nc"]
    _enable_ldw_opt()
    nc = bacc.Bacc(
        "TRN2", target_bir_lowering=False, debug=False, num_devices=NCORES
    )
    xp = nc.dram_tensor("xp", (P, KT * N), F16, kind="ExternalInput").ap()
    wq = nc.dram_tensor("wq", (P, KT * 256), F16, kind="ExternalInput").ap()
    wk = nc.dram_tensor("wk", (P, KT * 256), F16, kind="ExternalInput").ap()
    wv = nc.dram_tensor("wv", (P, KT * DH), F16, kind="ExternalInput").ap()
    eye = nc.dram_tensor("eye", (DH, DH), F16, kind="ExternalInput").ap()
    oT = nc.dram_tensor("oT", (G * DH, N), F32, kind="ExternalOutput").ap()
    with tile.TileContext(nc) as tc:
        with ExitStack() as ctx:
            emit_kernel(ctx, tc, xp, wq, wk, wv, eye, oT)
    nc.compile()
    _CACHE["nc"] = nc
    return nc


def _pack_w(w):
    # (KT*128, width) -> sbuf layout [p, kt*width]
    kt, width = w.shape[0] // P, w.shape[1]
    return np.ascontiguousarray(
        w.reshape(kt, P, width).transpose(1, 0, 2).reshape(P, kt * width)
    )


def make_in_maps(x, Wq, Wkv, k_scale):
    x = np.asarray(x, dtype=np.float32)
    Wq = np.asarray(Wq, dtype=np.float32)
    Wkv = np.asarray(Wkv, dtype=np.float32)
    k_scale = np.asarray(k_scale, dtype=np.float32)
    # x packed to [p, ic, kt, c]: contiguous per-ic-chunk DMAs.
    xps = []
    for b in range(B):
        xT = x[b].T.astype(np.float16)  # (1024, 2048)
        xp = (
            xT.reshape(KT, P, NI, IC)
            .transpose(1, 2, 0, 3)
            .reshape(P, KT * N)
        )
        xps.append(np.ascontiguousarray(xp))
    in_maps = []
    for c in range(NCORES):
        b, kv = divmod(c, KV_HEADS)
        wk_base = Wkv[:, kv * DH : (kv + 1) * DH]
        wk_c = np.concatenate(
            [wk_base * k_scale[kv * G + j, 0][None, :] for j in range(G)], axis=1
        )
        in_maps.append(
            {
                "xp": xps[b],
                "wq": _pack_w(
                    Wq[:, kv * G * DH : (kv + 1) * G * DH].astype(np.float16)
                ),
                "wk": _pack_w(wk_c.astype(np.float16)),
                "wv": _pack_w(
                    Wkv[
                        :, KV_HEADS * DH + kv * DH : KV_HEADS * DH + (kv + 1) * DH
                    ].astype(np.float16)
                ),
                "eye": np.eye(DH, dtype=np.float16),
            }
        )
    return in_maps


def gather(results):
    out = np.empty((B, N, HEADS * DH), dtype=np.float32)
    for c in range(NCORES):
        b, kv = divmod(c, KV_HEADS)
        out[b, :, kv * G * DH : (kv + 1) * G * DH] = results[c]["oT"].T
    return out


def kernel(x, Wq, Wkv, k_scale, _trace=False):
    nc = build()
    in_maps = make_in_maps(x, Wq, Wkv, k_scale)
    res = run_bass_kernel_spmd(
        nc, in_maps, core_ids=list(range(NCORES)), trace=_trace
    )
    out = gather(res.results)
    if _trace:
        kernel.last_result = res
    return out


# revision 32
# speedup vs baseline: 1.2401x; 1.0277x over previous
"""Trainium2 Bass kernel for grouped-query attention with qk-norm.

Problem (hardcoded): x(2,2048,1024) @ Wq(1024,1024) / Wkv(1024,512),
16 query heads, 4 kv heads, head_dim 64, k_scale(16,1,64) applied to the
group-broadcast k. Output (2,2048,1024).

Sharding: 8 cores = batch(2) x kv_heads(4). Each core computes its batch's
4 query heads against its kv head over the full 2048x2048 score matrix.

The kernel is scheduled around the Scalar (ACT) engine: softmax exp over
4 heads x 2048^2 scores is 16.8M activations ~= 143us of ACT busy time,
the per-core critical path. Levers used to stay at/below that floor:
- Scores computed transposed (S^T: keys on partitions) so exp(S^T) feeds
  PV directly as the moving operand; softmax skips max-subtraction
  (inputs bounded) and normalizes after PV via an appended ones-row in V.
- Inputs x/W and the q/k/v activations are fp16 (rel err ~5e-4).
- A subset of exp tiles runs on the Vector engine via a Schraudolph exp2
  bit trick (i16 = round(s*scale*log2e*1024 + 15*1024 - 60), bitcast to
  fp16; rel err ~1.7% on those keys only -> ~6e-3 overall at phi~0.23,
  against a 2e-2 gate), relieving ACT. Extra tiles go to the
  projection-heavy first block where ACT would outpace the PE anyway.
- Host packs x/Wq/Wk/Wv into the exact SBUF layouts so every input DMA
  is contiguous (4KB lines; strided layouts measured ~23GB/s vs ~180+
  contiguous), spread over the three DMA-capable queues JIT.
- Projection chain groups interleave their accumulation matmuls over kt
  so the PE keeps high duty while x streams in (HAM stays warm).
- All transient PSUM (scores, projections, warmups, v-transposes) shares
  one 3-slot ring (6 banks) + 2 banks for the PV accumulators, so a
  Vector-engine exp tile never blocks the Scalar engine's next tile.
- Each 512-query block drains, reciprocals (via a DRAM-bounce respread)
  and writes out inline under the next block's exp; the last block
  normalizes straight from PSUM with a 1-row iterative reciprocal.
"""

import os
from contextlib import ExitStack

import numpy as np

import concourse.bacc as bacc
import concourse.mybir as mybir
import concourse.tile as tile
from concourse.bass_utils import run_bass_kernel_spmd

# Problem constants
B, N, DIM = 2, 2048, 1024
HEADS, KV_HEADS, DH = 16, 4, 64
G = HEADS // KV_HEADS  # query heads per kv head (4)
NCORES = 8
P = 128
KT = DIM // P  # 8 contraction tiles over dim
IC = 512  # query-chunk width
NI = N // IC  # 4
NJ = N // P  # 16 key tiles
NW = NJ // NI  # 4 key tiles per projection wave
SCALE = DH**-0.5

F32 = mybir.dt.float32
F16 = mybir.dt.float16
I16 = mybir.dt.int16

# Schraudolph exp2 offload to the Vector engine. NS0 tiles (of 16) in the
# first block, NS elsewhere.
NSCH0 = int(os.environ.get("KERNEL_NSCH0", "0"))
NSCH = int(os.environ.get("KERNEL_NSCH", "3"))
SCH_C = float(os.environ.get("KERNEL_SCH_C", "60.0"))
SCH_MULT = SCALE * np.log2(np.e) * 1024.0
SCH_ADD = 15.0 * 1024.0 - SCH_C


def emit_kernel(ctx, tc, xp, wq, wk, wv, eye, oT):
    nc = tc.nc
    Exp = mybir.ActivationFunctionType.Exp
    mult = mybir.AluOpType.mult
    add = mybir.AluOpType.add

    wpool = ctx.enter_context(tc.tile_pool(name="w", bufs=1))
    qkpool = ctx.enter_context(tc.tile_pool(name="qk", bufs=1))
    ptpool = ctx.enter_context(tc.tile_pool(name="pt", bufs=6))
    npool = ctx.enter_context(tc.tile_pool(name="norm", bufs=2))
    # PSUM budget (8 banks of 2KB/partition):
    #   shared ring (scores/proj/warmup/vT) 3 x [128,1024] f32 = 6 banks
    #   o_ps (PV accumulators)              2 x [65,512]   f32 = 2 banks
    apsum = ctx.enter_context(tc.tile_pool(name="ap", bufs=3, space="PSUM"))
    opool = ctx.enter_context(tc.tile_pool(name="op", bufs=1, space="PSUM"))

    # --- persistent SBUF tensors ---
    ones_sb = wpool.tile([P, DH], F32, tag="ones")
    eye_sb = wpool.tile([DH, DH], F16, tag="eye")
    qT = [qkpool.tile([P, N], F16, name=f"qT{hp}", tag=f"qT{hp}") for hp in range(2)]
    kkT = [qkpool.tile([P, N], F16, name=f"kkT{hp}", tag=f"kkT{hp}") for hp in range(2)]
    vaug = qkpool.tile([P, NJ * (DH + 1)], F16, tag="vaug")
    wq_sb = qkpool.tile([P, KT * 256], F16, tag="wq")
    wk_sb = qkpool.tile([P, KT * 256], F16, tag="wk")
    wv_sb = qkpool.tile([P, KT * DH], F16, tag="wv")
    # x resident in SBUF, laid out [p, ic, kt, c] so each ic chunk is one
    # contiguous DMA from the host-packed xp.
    xts = qkpool.tile([P, KT * N], F16, tag="xt")  # 4MB
    vT_sb = qkpool.tile([DH, N], F16, tag="vT")

    def xsl(ic, kt):
        base = ic * KT * IC + kt * IC
        return xts[:, base : base + IC]

    nc.any.memset(vaug[:], 1.0)
    nc.any.memset(ones_sb[:], 1.0)
    warm = qkpool.tile([1, 1], F32, tag="warm")
    nc.scalar.activation(warm[:], ones_sb[0:1, 0:1], Exp)

    sums_d = nc.dram_tensor("sums_d", (G, N), F32, kind="ExternalOutput").ap()
    rec_d = nc.dram_tensor("rec_d", (G, N), F32, kind="ExternalOutput").ap()
    o_acc = [
        npool.tile([DH + 1, N], F32, name=f"oacc{h}", tag=f"oacc{h}", bufs=1)
        for h in range(G)
    ]
    rec_row = [
        npool.tile([1, N], F32, name=f"recrow{h}", tag=f"recrow{h}", bufs=1)
        for h in range(G)
    ]

    # --- input DMAs: all fully contiguous (host-packed), spread JIT ---
    #   gpsimd: x-ic0 (two halves, gates first kk/q chains), x-ic2
    #   sync:   wk, x-ic1, x-ic3
    #   scalar: wq, wv  (scalar queue frees well before the first exp)
    XC = KT * IC  # 4096 columns per ic chunk
    nc.sync.dma_start(eye_sb[:], eye[:, :])
    # Queues sustain only ~55GB/s each; chunk the first block's inputs so
    # the kk/q chains pipeline with arrival instead of waiting for a
    # single completion semaphore.
    HW = KT * 128  # half of a packed weight
    nc.sync.dma_start(wk_sb[:, 0:HW], wk[:, 0:HW])
    nc.scalar.dma_start(wq_sb[:, 0:HW], wq[:, 0:HW])
    nc.gpsimd.dma_start(xts[:, 0 : XC // 4], xp[:, 0 : XC // 4])
    nc.sync.dma_start(wk_sb[:, HW:], wk[:, HW:])
    nc.scalar.dma_start(wq_sb[:, HW:], wq[:, HW:])
    nc.gpsimd.dma_start(xts[:, XC // 4 : XC // 2], xp[:, XC // 4 : XC // 2])
    nc.scalar.dma_start(xts[:, XC // 2 : 3 * XC // 4], xp[:, XC // 2 : 3 * XC // 4])
    nc.sync.dma_start(xts[:, 3 * XC // 4 : XC], xp[:, 3 * XC // 4 : XC])
    nc.scalar.dma_start(wv_sb[:], wv[:, :])
    nc.gpsimd.dma_start(xts[:, XC : 2 * XC], xp[:, XC : 2 * XC])
    nc.sync.dma_start(xts[:, 2 * XC : 3 * XC], xp[:, 2 * XC : 3 * XC])
    nc.gpsimd.dma_start(xts[:, 3 * XC : 4 * XC], xp[:, 3 * XC : 4 * XC])

    # Dummy matmuls during the initial DMA wait keep the PE HAM activity
    # monitor busy so real projections start at 2.4GHz instead of 1.2.
    for _ in range(32):
        wt = apsum.tile([DH, IC], F32, tag="s", name="wt")
        nc.tensor.matmul(
            wt[:, 0:DH], ones_sb[:, 0:DH], ones_sb[:, 0:DH], start=True, stop=True
        )

    # --- projection waves (emitted JIT inside the attention loop) ---
    # A group of chains interleaves its matmuls over kt so the PE keeps
    # high duty while x chunks stream in.
    def proj_group(chains, ic, pool=None, tag="s"):
        csl = slice(ic * IC, (ic + 1) * IC)
        pool = pool or apsum
        pss = [
            pool.tile([rows, IC], F32, tag=tag, name="pjt")
            for (dst, w_sb, c0, rows) in chains
        ]
        for kt in range(KT):
            for (dst, w_sb, c0, rows), ps in zip(chains, pss):
                nc.tensor.matmul(
                    ps[:],
                    w_sb[:, kt * 256 + c0 : kt * 256 + c0 + rows]
                    if rows == P
                    else w_sb[:, kt * DH : (kt + 1) * DH],
                    xsl(ic, kt),
                    start=(kt == 0),
                    stop=(kt == KT - 1),
                )
        for (dst, w_sb, c0, rows), ps in zip(chains, pss):
            nc.vector.tensor_copy(dst[:, csl], ps[:])

    def kk_wave(ic, hps):
        proj_group([(kkT[hp], wk_sb, hp * 128, P) for hp in hps], ic)

    def q_wave(ic, hps):
        proj_group([(qT[hp], wq_sb, hp * 128, P) for hp in hps], ic)

    def v_wave(ic):
        proj_group([(vT_sb, wv_sb, 0, DH)], ic)
        # all 4 key-tile transposes into one psum tile, one strided copy out
        pv = apsum.tile([P, NW, DH], F16, tag="s", name="pvt")
        for w in range(NW):
            jt = NW * ic + w
            nc.tensor.transpose(
                pv[:, w, :], vT_sb[:, jt * P : (jt + 1) * P], eye_sb[:]
            )
        dst = vaug[:, NW * ic * (DH + 1) : (NW * ic + NW) * (DH + 1)]
        nc.vector.tensor_copy(
            dst.rearrange("p (w c) -> p w c", c=DH + 1)[:, :, 0:DH], pv[:]
        )

    # --- attention primitives ---
    def qk_mm(hp, ic, jt):
        csl = slice(ic * IC, (ic + 1) * IC)
        st = apsum.tile([P, 2 * IC], F32, tag="s", name="st")
        for half in range(2):
            rsl = slice(half * 64, half * 64 + 64)
            nc.tensor.matmul(
                st[:, half * IC : (half + 1) * IC],
                kkT[hp][rsl, jt * P : (jt + 1) * P],
                qT[hp][rsl, csl],
                start=True,
                stop=True,
                tile_position=(half * 64, 0),
            )
        return st

    def exp_tile(st, use_dve):
        pt = ptpool.tile([P, 2 * IC], F16, tag="pt")
        if use_dve:
            nc.vector.tensor_scalar(
                pt[:].bitcast(I16), st[:], SCH_MULT, SCH_ADD, mult, add
            )
        else:
            nc.scalar.activation(pt[:], st[:], Exp, scale=SCALE)
        return pt

    def pv_mm(o_ps, jt, pt):
        for half in range(2):
            nc.tensor.matmul(
                o_ps[:, half * IC : (half + 1) * IC],
                vaug[:, jt * (DH + 1) : (jt + 1) * (DH + 1)],
                pt[:, half * IC : (half + 1) * IC],
                start=(jt == 0),
                stop=(jt == NJ - 1),
            )

    def recip_chunk(h, ic):
        # DVE reciprocal on a 1-row AP is iterative (~8cyc/elem on one
        # lane); on (128,4) it is ~100x cheaper. The sums row is respread
        # across partitions via a DRAM bounce (DMA cannot repartition
        # within SBUF). Latency hides under the next block.
        csl = slice(ic * IC, (ic + 1) * IC)
        sums_t = npool.tile([P, 4], F32, tag="sums_t", bufs=2)
        rec_t = npool.tile([P, 4], F32, tag="rec_t", bufs=2)
        nc.sync.dma_start(sums_d[h : h + 1, csl], o_acc[h][DH : DH + 1, csl])
        nc.sync.dma_start(
            sums_t[:], sums_d[h : h + 1, csl].rearrange("o (p f) -> (o p) f", p=P)
        )
        nc.vector.reciprocal(rec_t[:], sums_t[:])
        nc.sync.dma_start(
            rec_d[h : h + 1, csl].rearrange("o (p f) -> (o p) f", p=P), rec_t[:]
        )
        nc.sync.dma_start(rec_row[h][0:1, csl], rec_d[h : h + 1, csl])

    def normalize_chunk(h, ic, src):
        # GpSimd broadcasts the reciprocal row across partitions (PE-free).
        csl = slice(ic * IC, (ic + 1) * IC)
        bc = npool.tile([DH, IC], F32, name="bcg", tag="bcg", bufs=4)
        nc.gpsimd.partition_broadcast(bc[:], rec_row[h][0:1, csl])
        fin = npool.tile([DH, IC], F32, tag="fin", bufs=4)
        nc.vector.tensor_tensor(fin[:], src, bc[:], mult)
        nc.sync.dma_start(oT[h * DH : (h + 1) * DH, csl], fin[:])

    def drain_block(hp, ic, o_ps, last):
        csl = slice(ic * IC, (ic + 1) * IC)
        if last:
            # Shortest exposed tail: iterative reciprocal directly on the
            # 1-row psum sums (~3.3us each on one lane, but no 4-hop DRAM
            # bounce latency), normalize straight from PSUM.
            for half in range(2):
                h = 2 * hp + half
                nc.vector.reciprocal(
                    rec_row[h][0:1, csl],
                    o_ps[DH : DH + 1, half * IC : (half + 1) * IC],
                )
                normalize_chunk(h, ic, o_ps[0:DH, half * IC : (half + 1) * IC])
        else:
            for half in range(2):
                h = 2 * hp + half
                nc.vector.tensor_copy(
                    o_acc[h][:, csl], o_ps[:, half * IC : (half + 1) * IC]
                )
                recip_chunk(h, ic)
                normalize_chunk(h, ic, o_acc[h][0:DH, csl])

    def sch_set(n):
        if n <= 0:
            return set()
        step = NJ // n
        return {NJ - 1 - i * step for i in range(n)}

    # --- main loop: ACT-centric pipeline ---
    # Block (0,0) carries the projection waves JIT: kk chunk w must precede
    # QK of j-tiles 4w.., v chunk w must precede PV of j-tile 4w. hp=1
    # projections are deferred into later blocks' PE slack. PV is software-
    # pipelined ACROSS block boundaries (lag 2): the next block's first QKs
    # issue before the previous block's last PVs, so the exp stream never
    # stalls at a block transition.
    proj_group([(kkT[0], wk_sb, 0, P), (qT[0], wq_sb, 0, P)], 0)
    blocks = [(hp, ic) for hp in range(2) for ic in range(NI)]
    fasttail = os.environ.get("KERNEL_FASTTAIL", "1") == "1"
    pend = []  # (bi, jt, pt) awaiting PV
    o_psd = {}

    def flush_one():
        bi0, j0, pt0 = pend.pop(0)
        if j0 == 0:
            if bi0 > 0:
                ph, pi = blocks[bi0 - 1]
                drain_block(ph, pi, o_psd.pop(bi0 - 1), last=False)
            o_psd[bi0] = opool.tile(
                [DH + 1, 2 * IC], F32, name="ops", tag="ops", bufs=1
            )
        pv_mm(o_psd[bi0], j0, pt0)

    for bi, (hp, ic) in enumerate(blocks):
        first_block = bi == 0
        last_block = bi == len(blocks) - 1
        sch = sch_set(NSCH0 if first_block else NSCH)
        if last_block and sch:
            # keep the final exps on ACT so the tail isn't gated on the
            # slower Vector-engine exp
            sch = {jt - 3 for jt in sch}
        for jt in range(NJ):
            if first_block:
                if jt % NW == 0 and jt > 0:
                    kk_wave(jt // NW, [0])
                if jt == 2:
                    kk_wave(0, [1])
            st = qk_mm(hp, ic, jt)
            if first_block and jt % NW == 0:
                v_wave(jt // NW)
            if jt == 8 and not last_block:
                # next block's q projection, mid-block so the score
                # pipeline absorbs it instead of stalling at the boundary
                nhp, nic = (hp, ic + 1) if ic < NI - 1 else (1, 0)
                q_wave(nic, [nhp])
            if jt == 12 and hp == 0 and 0 < ic:
                # spread the deferred hp=1 kk projections over blocks
                # (0,1)-(0,3) instead of overloading the first block
                kk_wave(ic, [1])
            pend.append((bi, jt, exp_tile(st, jt in sch)))
            while len(pend) > 2:
                flush_one()
    while pend:
        flush_one()
    drain_block(1, NI - 1, o_psd.pop(len(blocks) - 1), last=fasttail)


_CACHE = {}


def _enable_ldw_opt():
    # The concourse walrus wrapper hardcodes --enable-ldw-opt=false; with it
    # off every matmul pays an exposed LDWEIGHTS (~120ns x ~560 matmuls).
    # Opt in to the optimized weight-load path for this kernel; correctness
    # is checked end-to-end against the reference output.
    # Tried: breaks walrus codegen (visitInstLdweights assertion) in this
    # compiler build — that is why concourse pins it false. Off by default.
    if os.environ.get("KERNEL_LDW_OPT", "0") != "1":
        return
    import concourse.bass_utils as bu

    orig = bu.run_command
    if getattr(orig, "_ldw_patched", False):
        return

    def run_command_ldw(cmd, *a, **kw):
        if isinstance(cmd, list):
            cmd = [
                "--enable-ldw-opt=true" if c == "--enable-ldw-opt=false" else c
                for c in cmd
            ]
        return orig(cmd, *a, **kw)

    run_command_ldw._ldw_patched = True
    bu.run_command = run_command_ldw


def build():
    if "nc" in _CACHE:
        return _CACHE["nc"]
    _enable_ldw_opt()
    nc = bacc.Bacc(
        "TRN2", target_bir_lowering=False, debug=False, num_devices=NCORES
    )
    xp = nc.dram_tensor("xp", (P, KT * N), F16, kind="ExternalInput").ap()
    wq = nc.dram_tensor("wq", (P, KT * 256), F16, kind="ExternalInput").ap()
    wk = nc.dram_tensor("wk", (P, KT * 256), F16, kind="ExternalInput").ap()
    wv = nc.dram_tensor("wv", (P, KT * DH), F16, kind="ExternalInput").ap()
    eye = nc.dram_tensor("eye", (DH, DH), F16, kind="ExternalInput").ap()
    oT = nc.dram_tensor("oT", (G * DH, N), F32, kind="ExternalOutput").ap()
    with tile.TileContext(nc) as tc:
        with ExitStack() as ctx:
            emit_kernel(ctx, tc, xp, wq, wk, wv, eye, oT)
    nc.compile()
    _CACHE["nc"] = nc
    return nc


def _pack_w(w):
    # (KT*128, width) -> sbuf layout [p, kt*width]
    kt, width = w.shape[0] // P, w.shape[1]
    return np.ascontiguousarray(
        w.reshape(kt, P, width).transpose(1, 0, 2).reshape(P, kt * width)
    )


def make_in_maps(x, Wq, Wkv, k_scale):
    x = np.asarray(x, dtype=np.float32)
    Wq = np.asarray(Wq, dtype=np.float32)
    Wkv = np.asarray(Wkv, dtype=np.float32)
    k_scale = np.asarray(k_scale, dtype=np.float32)
    # x packed to [p, ic, kt, c]: contiguous per-ic-chunk DMAs.
    xps = []
    for b in range(B):
        xT = x[b].T.astype(np.float16)  # (1024, 2048)
        xp = (
            xT.reshape(KT, P, NI, IC)
            .transpose(1, 2, 0, 3)
            .reshape(P, KT * N)
        )
        xps.append(np.ascontiguousarray(xp))
    in_maps = []
    for c in range(NCORES):
        b, kv = divmod(c, KV_HEADS)
        wk_base = Wkv[:, kv * DH : (kv + 1) * DH]
        wk_c = np.concatenate(
            [wk_base * k_scale[kv * G + j, 0][None, :] for j in range(G)], axis=1
        )
        in_maps.append(
            {
                "xp": xps[b],
                "wq": _pack_w(
                    Wq[:, kv * G * DH : (kv + 1) * G * DH].astype(np.float16)
                ),
                "wk": _pack_w(wk_c.astype(np.float16)),
                "wv": _pack_w(
                    Wkv[
                        :, KV_HEADS * DH + kv * DH : KV_HEADS * DH + (kv + 1) * DH
                    ].astype(np.float16)
                ),
                "eye": np.eye(DH, dtype=np.float16),
            }
        )
    return in_maps


def gather(results):
    out = np.empty((B, N, HEADS * DH), dtype=np.float32)
    for c in range(NCORES):
        b, kv = divmod(c, KV_HEADS)
        out[b, :, kv * G * DH : (kv + 1) * G * DH] = results[c]["oT"].T
    return out


def kernel(x, Wq, Wkv, k_scale, _trace=False):
    nc = build()
    in_maps = make_in_maps(x, Wq, Wkv, k_scale)
    res = run_bass_kernel_spmd(
        nc, in_maps, core_ids=list(range(NCORES)), trace=_trace
    )
    out = gather(res.results)
    if _trace:
        kernel.last_result = res
    return out
